# revision 24
# baseline (speedup 1.0000x reference)
# Causal self-attention (B=8, T=1024, C=1024, H=16, D=64) on 8 trn2 NeuronCores.
# Sharding: data-parallel over batch — core i computes batch element i entirely
# (weights replicated, no collectives).
#
# Per-core pipeline (all matmuls bf16 inputs, fp32 PSUM accumulation):
#   0. x [T,C] --cast-dma--> bf16, PE-transpose -> xT [C,T] (8 chunks of [128, T])
#   1. v[t,j] = lhsT=xT chunk, rhs=W_v slices (+bias ones-row mm), stored per-head
#      with a ones column (width 65) so attn@v also yields the softmax denominator.
#   2. interleaved per head-pair hp: qkT[j,t] for jt=hp and 8+hp (lhsT=W column
#      slice, rhs=xT), then attention for heads 2hp, 2hp+1:
#      per (head, 512-query tile): scores sT[j,i] (K=64, causally trimmed),
#      diag-block mask add (DVE), exp on ACT (scale=1/8) -> pT bf16,
#      attn@v (M=65; row 64 = l = sum_j p), reciprocal_approx from PSUM,
#      K=1 matmul broadcast, ACT copy, DVE mult -> oT [c_in, t] bf16.
#   3. out[t,c] : lhsT=oT chunk, rhs=W_out (+bias ones-row mm) -> DMA to DRAM.
#
# W_qkv is DMA'd in column slices (per jt / per v-half) so matmuls start as
# soon as their slice lands; x is DMA'd first.

import numpy as np
from contextlib import ExitStack

import concourse.bass as bass
import concourse.bacc as bacc
import concourse.mybir as mybir
import concourse.tile as tile
from concourse import bass_utils
from concourse.masks import make_identity

FP32 = mybir.dt.float32
BF16 = mybir.dt.bfloat16

B, T, C = 8, 1024, 1024
H, D = 16, 64
N_CORES = 8
MASK_VAL = -1e4  # pre-scale additive mask; exp(0.125 * (s + MASK_VAL)) == 0.0
CCH = C // 128   # 8 contraction chunks of 128
TCH = T // 128   # 8 token chunks of 128


def build_nc():
    nc = bacc.Bacc("TRN2", debug=False, num_devices=N_CORES)

    x_d = nc.dram_tensor("x_b", [T, C], FP32, kind="ExternalInput").ap()
    wq_d = nc.dram_tensor("w_qkv", [C, 3 * C], FP32, kind="ExternalInput").ap()
    bq_d = nc.dram_tensor("b_qkv", [1, 3 * C], FP32, kind="ExternalInput").ap()
    wo_d = nc.dram_tensor("w_out", [C, C], FP32, kind="ExternalInput").ap()
    bo_d = nc.dram_tensor("b_out", [1, C], FP32, kind="ExternalInput").ap()
    out_d = nc.dram_tensor("out_b", [T, C], FP32, kind="ExternalOutput").ap()

    with tile.TileContext(nc) as tc, ExitStack() as ctx:
        consts = ctx.enter_context(tc.tile_pool(name="consts", bufs=1))
        wpool = ctx.enter_context(tc.tile_pool(name="weights", bufs=1))
        apool = ctx.enter_context(tc.tile_pool(name="acts", bufs=1))
        ppool = ctx.enter_context(tc.tile_pool(name="ppool", bufs=8))
        lpool = ctx.enter_context(tc.tile_pool(name="lpool", bufs=2))
        rbpool = ctx.enter_context(tc.tile_pool(name="rbpool", bufs=2))
        outs = ctx.enter_context(tc.tile_pool(name="outs", bufs=2))
        # PSUM: P1 generic [128,512] (projections), P2 scores+bcast, P3 attn out
        P1 = ctx.enter_context(tc.tile_pool(name="P1", bufs=2, space="PSUM"))
        P2 = ctx.enter_context(tc.tile_pool(name="P2", bufs=2, space="PSUM"))
        P3 = ctx.enter_context(tc.tile_pool(name="P3", bufs=2, space="PSUM"))

        # ---- constants (tiny DMAs first) ----
        bqkv_sb = consts.tile([1, 3 * C], BF16, tag="bqkv")
        nc.gpsimd.dma_start(out=bqkv_sb, in_=bq_d)
        bout_sb = consts.tile([1, C], BF16, tag="bout")
        nc.gpsimd.dma_start(out=bout_sb, in_=bo_d)

        identity = consts.tile([128, 128], BF16, tag="identity")
        make_identity(nc, identity)
        # diag_mask[jj, ii] = 0 if ii >= jj else MASK_VAL  (valid = key <= query)
        diag_mask = consts.tile([128, 128], FP32, tag="diag_mask")
        nc.gpsimd.memset(diag_mask, 0.0)
        nc.gpsimd.affine_select(
            out=diag_mask, in_=diag_mask,
            compare_op=mybir.AluOpType.is_ge, fill=MASK_VAL,
            base=0, channel_multiplier=-1, pattern=[[1, 128]],
        )
        ones_row = consts.tile([1, T], BF16, tag="ones_row")
        nc.vector.memset(ones_row, 1.0)
        ones64 = consts.tile([1, 64], FP32, tag="ones64")
        nc.vector.memset(ones64, 1.0)
        # b_qkv for the q/k part transposed to per-partition layout [128, 16]
        bqT = consts.tile([128, 16], FP32, tag="bqT")
        nc.gpsimd.dma_start(
            out=bqT,
            in_=bq_d[:, 0:2 * C].rearrange("x (jt p) -> p (x jt)", p=128))

        # ---- persistent activations ----
        xT = [apool.tile([128, T], BF16, tag=f"xT{cc}", name=f"xT{cc}")
              for cc in range(CCH)]
        qkT = [apool.tile([128, T], BF16, tag=f"qkT{jt}", name=f"qkT{jt}")
               for jt in range(16)]
        vp = [apool.tile([128, H * (D + 1)], BF16, tag=f"vp{t_}", name=f"vp{t_}")
              for t_ in range(TCH)]
        oT = [apool.tile([128, T], BF16, tag=f"oT{hc}", name=f"oT{hc}")
              for hc in range(CCH)]

        # ---- phase 0: x load (cast bf16, first in DMA queue) + PE transpose ----
        with tc.tile_pool(name="xstage", bufs=4) as xstage:
            # x chunks and v-part weight slices interleaved in the DMA queue
            # so v-projection can start as early as possible.
            xs_all = []
            wv_col = [wpool.tile([128, CCH, 512], BF16, tag=f"wv{j}", name=f"wv{j}")
                      for j in range(2)]

            def _x_dma(ti):
                xs = xstage.tile([128, C], BF16, tag="xs", name=f"xs{ti}")
                nc.gpsimd.dma_start(out=xs, in_=x_d[ti * 128:(ti + 1) * 128, :])
                xs_all.append(xs)

            def _wv_dma(jvt):
                src = wq_d[:, 2 * C + jvt * 512: 2 * C + (jvt + 1) * 512]
                nc.gpsimd.dma_start(
                    out=wv_col[jvt], in_=src.rearrange("(cc p) j -> p cc j", p=128))

            _x_dma(0); _x_dma(1)
            _wv_dma(0)
            for ti in range(2, TCH):
                _x_dma(ti)
            _wv_dma(1)
            # q/k parts per jt, in head-pair consumption order
            wq_col = [None] * 16
            for hp in range(8):
                for jt in (hp, 8 + hp):
                    t_ = wpool.tile([128, CCH, 128], BF16,
                                    tag=f"wq{jt}", name=f"wq{jt}")
                    src = wq_d[:, jt * 128:(jt + 1) * 128]
                    nc.gpsimd.dma_start(
                        out=t_, in_=src.rearrange("(cc p) j -> p cc j", p=128))
                    wq_col[jt] = t_
            # out projection weights, one DMA (needed last)
            wo_col = wpool.tile([128, CCH, C], BF16, tag="wo", name="wo")
            nc.gpsimd.dma_start(
                out=wo_col, in_=wo_d.rearrange("(cc p) j -> p cc j", p=128))

            # transposes: x[t,c] 128x128 blocks -> xT[c,t]
            for ti in range(TCH):
                for cc in range(CCH):
                    pt = P3.tile([128, 128], BF16, tag="po", name="tp")
                    nc.tensor.transpose(
                        out=pt, in_=xs_all[ti][:, cc * 128:(cc + 1) * 128],
                        identity=identity)
                    nc.vector.tensor_copy(
                        out=xT[cc][:, ti * 128:(ti + 1) * 128], in_=pt)

            # broadcast biases across partitions once (K=1 matmul + copy):
            # kills the per-tile ones-row bias matmuls in v/out projections.
            bvb = consts.tile([128, C], FP32, tag="bvb")
            bob = consts.tile([128, C], FP32, tag="bob")
            for half in range(2):
                sl = slice(half * 512, (half + 1) * 512)
                pb = P1.tile([128, 512], FP32, tag="p1", name="pbias")
                nc.tensor.matmul(
                    out=pb, lhsT=ones_row[0:1, 0:128],
                    rhs=bqkv_sb[0:1, 2 * C + half * 512:2 * C + (half + 1) * 512],
                    start=True, stop=True)
                nc.vector.tensor_copy(out=bvb[:, sl], in_=pb)
                pb2 = P1.tile([128, 512], FP32, tag="p1", name="pbias2")
                nc.tensor.matmul(out=pb2, lhsT=ones_row[0:1, 0:128],
                                 rhs=bout_sb[0:1, sl], start=True, stop=True)
                nc.vector.tensor_copy(out=bob[:, sl], in_=pb2)

            # ---- v projection (natural layout, into vp with stride 65) ----
            for ti in range(TCH):
                vcol = vp[ti].rearrange("p (h d) -> p h d", h=H)
                nc.vector.memset(vcol[:, :, D:D + 1], 1.0)
                for jvt in range(2):
                    ps = P1.tile([128, 512], FP32, tag="p1", name="psv")
                    for cc in range(CCH):
                        nc.tensor.matmul(
                            out=ps,
                            lhsT=xT[cc][:, ti * 128:(ti + 1) * 128],
                            rhs=wv_col[jvt][:, cc, :],
                            start=(cc == 0), stop=(cc == CCH - 1))
                    nc.vector.tensor_tensor(
                        out=vcol[:, jvt * 8:(jvt + 1) * 8, 0:D],
                        in0=ps.rearrange("p (h d) -> p h d", h=8),
                        in1=bvb[:, jvt * 512:(jvt + 1) * 512].rearrange(
                            "p (h d) -> p h d", h=8),
                        op=mybir.AluOpType.add)

            # ---- interleaved: qk projection + attention, software-pipelined
            # (qk groups of pair hp+1 are emitted between the attention units
            # of pair hp so PE always has dense independent work queued) ----
            def qk_group(jt, half):
                sl = slice(half * 512, (half + 1) * 512)
                ps = P1.tile([128, 512], FP32, tag="p1", name="psqk")
                for cc in range(CCH):
                    nc.tensor.matmul(
                        out=ps,
                        lhsT=wq_col[jt][:, cc, :],
                        rhs=xT[cc][:, sl],
                        start=(cc == 0), stop=(cc == CCH - 1))
                # bias folded into the copy (per-partition scalar)
                nc.vector.tensor_scalar_add(
                    out=qkT[jt][:, sl], in0=ps, scalar1=bqT[:, jt:jt + 1])

            qk_queue = [(jt, half) for hp in range(8)
                        for jt in (hp, 8 + hp) for half in range(2)]
            for g in qk_queue[:4]:
                qk_group(*g)
            qk_pos = 4

            for hp in range(8):
                # attention for the head pair, jointly: both heads' score
                # matmuls are adjacent (K=64 at partition bases 0/64 -> PE
                # row-groups can overlap) and share one batched exp.
                h0, h1 = 2 * hp, 2 * hp + 1
                qk_q, qk_k = qkT[hp], qkT[8 + hp]
                for it in range(2):
                    njc = 4 * (it + 1)
                    po2 = [P3.tile([65, 512], FP32, tag="po", name=f"po{hx}")
                           for hx in range(2)]
                    for jc in range(njc):
                        s0 = max(0, jc * 128 - it * 512)
                        ps = P2.tile([128, 2, 512], FP32, tag="ps", name="pss")
                        for hx, h in enumerate((h0, h1)):
                            prow = slice(hx * 64, hx * 64 + 64)
                            nc.tensor.matmul(
                                out=ps[:, hx, s0:512],
                                lhsT=qk_k[prow, jc * 128:(jc + 1) * 128],
                                rhs=qk_q[prow, it * 512 + s0:(it + 1) * 512],
                                start=True, stop=True)
                        if jc >= it * 4:  # diagonal block cols [s0, s0+128)
                            for hx in range(2):
                                nc.vector.tensor_tensor(
                                    out=ps[:, hx, s0:s0 + 128],
                                    in0=ps[:, hx, s0:s0 + 128],
                                    in1=diag_mask, op=mybir.AluOpType.add)
                        pT = ppool.tile([128, 2, 512], BF16, tag="pT", name="pT")
                        nc.scalar.activation(
                            out=pT[:, :, s0:512], in_=ps[:, :, s0:512],
                            func=mybir.ActivationFunctionType.Exp, scale=0.125)
                        # PE filler while ACT computes the exp: one qk group
                        if jc % 2 == 1 and qk_pos < len(qk_queue):
                            qk_group(*qk_queue[qk_pos])
                            qk_pos += 1
                        for hx, h in enumerate((h0, h1)):
                            hsl = slice(h * (D + 1), h * (D + 1) + D + 1)
                            nc.tensor.matmul(
                                out=po2[hx][0:65, s0:512],
                                lhsT=vp[jc][:, hsl],
                                rhs=pT[:, hx, s0:512],
                                start=(jc == 0), stop=(jc == njc - 1),
                                skip_group_check=True)
                    # normalize: row 64 of po = l = sum_j p
                    for hx in range(2):
                        po = po2[hx]
                        prow = slice(hx * 64, hx * 64 + 64)
                        l_sb = lpool.tile([1, 512], FP32, tag="l", name="l")
                        nc.scalar.copy(out=l_sb, in_=po[64:65, :])
                        plb = P2.tile([64, 512], FP32, tag="ps", name="plb")
                        nc.tensor.matmul(out=plb, lhsT=ones64, rhs=l_sb,
                                         start=True, stop=True)
                        rb = rbpool.tile([64, 512], FP32, tag="rb", name="rb")
                        nc.vector.reciprocal_approx_fast(out=rb, in_=plb)
                        nc.vector.tensor_tensor(
                            out=oT[hp][prow, it * 512:(it + 1) * 512],
                            in0=po[0:64, :], in1=rb, op=mybir.AluOpType.mult)

            # ---- output projection ----
            for ti in range(TCH):
                ot = outs.tile([128, C], FP32, tag="ot", name="ot")
                for half in range(2):
                    sl = slice(half * 512, (half + 1) * 512)
                    ps = P1.tile([128, 512], FP32, tag="p1", name="pso")
                    for hc in range(CCH):
                        nc.tensor.matmul(
                            out=ps,
                            lhsT=oT[hc][:, ti * 128:(ti + 1) * 128],
                            rhs=wo_col[:, hc, sl],
                            start=(hc == 0), stop=(hc == CCH - 1))
                    nc.vector.tensor_tensor(
                        out=ot[:, sl], in0=ps, in1=bob[:, sl],
                        op=mybir.AluOpType.add)
                nc.sync.dma_start(out=out_d[ti * 128:(ti + 1) * 128, :], in_=ot)

    nc.compile()
    nc.finalize()
    return nc


_CACHE = {}


def kernel(x, W_qkv, b_qkv, W_out, b_out):
    if "nc" not in _CACHE:
        _CACHE["nc"] = build_nc()
    nc = _CACHE["nc"]
    x = np.ascontiguousarray(np.asarray(x, dtype=np.float32))
    in_maps = [
        {
            "x_b": x[i],
            "w_qkv": np.ascontiguousarray(np.asarray(W_qkv, np.float32)),
            "b_qkv": np.ascontiguousarray(np.asarray(b_qkv, np.float32).reshape(1, -1)),
            "w_out": np.ascontiguousarray(np.asarray(W_out, np.float32)),
            "b_out": np.ascontiguousarray(np.asarray(b_out, np.float32).reshape(1, -1)),
        }
        for i in range(N_CORES)
    ]
    res = bass_utils.run_bass_kernel_spmd(nc, in_maps, core_ids=list(range(N_CORES)))
    return np.stack([r["out_b"] for r in res.results]).astype(np.float32)
